# revision 1
# baseline (speedup 1.0000x reference)
"""Segment mean-pool (BERT lattice embedding) Trainium2 Bass kernel.

Full-input contract: kernel(hidden[64,512,768] f32, word_ids[64,512] i32,
num_tokens=400) -> [64,400,768] f32.

Strategy: data-parallel over batch across 8 NeuronCores (8 samples each).
Per sample b the ragged segment mean  out[t] = mean_{s: wid[s]==t} hidden[s]
is computed as a matmul on the PE array:

    A_T[s, t] = (word_ids[b, s] == t)            one-hot, built on-device
    psum[t, :] = sum_j A_T[j-chunk].T @ hidden[b, j-chunk]
    out[t, h] = psum[t, h] * recip[b, t]         recip = 1/max(count,1)

All matmuls run in float32r (FP22-truncated fp32): full PE rate at even
N>=256, ~2e-4 relative error, and no dtype casts of the 100 MB activation
tensor. The per-word piece counts (reciprocals) are derived on host from
the 128 KB word_ids index tensor — index-side preprocessing, like the shard
layout transform; all heavy data stays on device.

Layouts are chosen for maximally contiguous DMA descriptors:
  - pieces:  partition p holds s = 4p+j  -> input reads are 12 KB/partition
    contiguous (segment-sum is invariant to how s is split into K-chunks)
  - words:   partition p holds t = 4p+m  -> all four output m-chunks land in
    one [100, 4, H] tile per sample, written as 12 KB/partition contiguous
    runs with no ragged 400-row tail

DMA ring assignment: inputs prefetch on the sync HWDGE ring (entire shard up
front — fits SBUF), outputs stream on the scalar HWDGE ring, so output
drains never queue behind the input prefetch.
"""

import numpy as np

B, S, H, T = 64, 512, 768, 400
N_CORES = 8
B_LOC = B // N_CORES  # samples per core
P = 128
J = S // P  # contraction chunks per sample
N0 = 384  # h-chunk split: two equal psum banks, balances the scale engines
M_CHUNKS = [(0, 128), (128, 128), (256, 128), (384, T - 384)]  # (t0, mw)
NM = len(M_CHUNKS)

_CACHED = {}


def build_program():
    """Build + compile the single-core Bass program (same NEFF on all cores)."""
    import concourse.bass as bass  # noqa: F401
    import concourse.mybir as mybir
    import concourse.tile as tile
    from concourse import bacc

    nc = bacc.Bacc(
        "TRN2",
        target_bir_lowering=False,
        debug=False,
        enable_asserts=False,
        num_devices=N_CORES,
    )
    f32 = mybir.dt.float32
    f32r = mybir.dt.float32r

    # float32r == fp32 bit layout; the PE truncates to FP22 on read. Declaring
    # the whole hidden/one-hot path float32r satisfies walrus's fp32r-producer
    # rule without any casts or extra copies.
    hidden_t = nc.dram_tensor("hidden", [B_LOC, S, H], f32r, kind="ExternalInput").ap()
    # word_ids host-prearranged as [P, B_LOC, J] fp32 (values < 400 are exact):
    # wid_pbj[p, b, j] = word_ids[b, 4p+j], the per-partition scalar for
    # piece-chunk j. tensor_scalar(is_equal) requires fp32 operands.
    wid_t = nc.dram_tensor("word_ids_pbj", [P, B_LOC, J], f32, kind="ExternalInput").ap()
    # Host-computed 1/max(count,1): recip_pbm[p, b, m] = recip[b, 128m+p]
    # (t >= 400 padded with 1.0).
    recip_t = nc.dram_tensor("recip_pbm", [P, B_LOC, NM], f32, kind="ExternalInput").ap()
    out_t = nc.dram_tensor("out", [B_LOC, T, H], f32, kind="ExternalOutput").ap()

    with tile.TileContext(nc) as tc:
        with tc.tile_pool(name="const", bufs=1) as const_pool, \
             tc.tile_pool(name="hidp", bufs=B_LOC) as hid_pool, \
             tc.tile_pool(name="aTp", bufs=3) as aT_pool, \
             tc.tile_pool(name="outp", bufs=4) as out_pool, \
             tc.tile_pool(name="psum", bufs=4, space="PSUM") as psum_pool:

            iota_t = const_pool.tile([P, T], f32, name="iota_t")
            nc.gpsimd.iota(
                iota_t,
                pattern=[[1, T]],
                base=0,
                channel_multiplier=0,
                allow_small_or_imprecise_dtypes=True,
            )

            wid_sb = const_pool.tile([P, B_LOC, J], f32, name="wid_sb")
            nc.sync.dma_start(out=wid_sb, in_=wid_t)
            recip_sb = const_pool.tile([P, B_LOC, NM], f32, name="recip_sb")
            nc.sync.dma_start(out=recip_sb, in_=recip_t)


            # Prefetch the whole input shard up front (fits in SBUF): the
            # input queue streams back-to-back from t=0 and compute is never
            # input-starved. One DMA per sample; 3 KB descriptors measured
            # faster end-to-end than 12 KB ones (12 KB exceeds the preferred
            # DMA packet size and starves the concurrent output stream).
            hids = []
            for b in range(B_LOC):
                hid = hid_pool.tile([P, J, H], f32r, name=f"hid{b}", tag="hid")
                src = hidden_t[b].rearrange("(j p) h -> p j h", p=P)
                if b == 0:
                    # First sample split per j-chunk so the first accumulation
                    # can start ~3 us earlier, as soon as chunk 0 lands.
                    for j in range(J):
                        nc.sync.dma_start(out=hid[:, j, :], in_=src[:, j, :])
                else:
                    nc.sync.dma_start(out=hid, in_=src)
                hids.append(hid)

            for b in range(B_LOC):
                hid = hids[b]
                aT = aT_pool.tile([P, J, T], f32r, name="aT", tag="aT")
                for j in range(J):
                    nc.vector.tensor_scalar(
                        aT[:, j, :],
                        iota_t,
                        wid_sb[:, b, j : j + 1],
                        None,
                        op0=mybir.AluOpType.is_equal,
                    )
                for mi, (t0, mw) in enumerate(M_CHUNKS):
                    ps0 = psum_pool.tile([P, N0], f32, name="ps0", tag="ps0")
                    ps1 = psum_pool.tile([P, H - N0], f32, name="ps1", tag="ps1")
                    for j in range(J):
                        nc.tensor.matmul(
                            ps0[:mw],
                            aT[:, j, t0 : t0 + mw],
                            hid[:, j, 0:N0],
                            start=(j == 0),
                            stop=(j == J - 1),
                        )
                    for j in range(J):
                        nc.tensor.matmul(
                            ps1[:mw],
                            aT[:, j, t0 : t0 + mw],
                            hid[:, j, N0:H],
                            start=(j == 0),
                            stop=(j == J - 1),
                        )

                    rec = recip_sb[:, b, mi : mi + 1]
                    om = out_pool.tile([P, H], f32, name="om", tag="om")
                    # out = psum * (1/count): ACT and DVE each take one chunk,
                    # both read PSUM directly.
                    nc.scalar.mul(om[:mw, 0:N0], ps0[:mw], rec[:mw])
                    nc.vector.tensor_scalar_mul(om[:mw, N0:H], ps1[:mw], rec[:mw])
                    # Per-m-chunk output DMA right after its scale: outputs
                    # start streaming ~10 us earlier than per-sample batching.
                    # Scalar HWDGE ring — separate FIFO from the input
                    # prefetch.
                    nc.scalar.dma_start(out=out_t[b, t0 : t0 + mw], in_=om[:mw])

    nc.compile()
    return nc


def _prep_in_maps(hidden, word_ids):
    hidden = np.ascontiguousarray(np.asarray(hidden), dtype=np.float32).reshape(B, S, H)
    wid = np.ascontiguousarray(np.asarray(word_ids), dtype=np.int32).reshape(B, S)

    # Per-word piece counts -> 1/max(count,1), padded to 512 words per sample.
    counts = np.zeros((B, P * NM), np.int64)
    rows = np.repeat(np.arange(B), S)
    np.add.at(counts, (rows, wid.reshape(-1)), 1)
    recip = (1.0 / np.maximum(counts, 1)).astype(np.float32)  # [B, 512]

    in_maps = []
    for i in range(N_CORES):
        sl = slice(i * B_LOC, (i + 1) * B_LOC)
        hs = np.ascontiguousarray(hidden[sl])
        ws = wid[sl]
        # [B_LOC, S] -> [P, B_LOC, J]: wid_pbj[p, b, j] = wid[b, 128j+p]
        wpbj = np.ascontiguousarray(
            ws.reshape(B_LOC, J, P).transpose(2, 0, 1).astype(np.float32)
        )
        # recip_pbm[p, b, m] = recip[b, 128m+p]
        rpbm = np.ascontiguousarray(recip[sl].reshape(B_LOC, NM, P).transpose(2, 0, 1))
        in_maps.append({"hidden": hs, "word_ids_pbj": wpbj, "recip_pbm": rpbm})
    return in_maps


def run(hidden, word_ids, trace=False, **trace_kwargs):
    from concourse import bass_utils

    if "nc" not in _CACHED:
        _CACHED["nc"] = build_program()
    nc = _CACHED["nc"]
    in_maps = _prep_in_maps(hidden, word_ids)
    res = bass_utils.run_bass_kernel_spmd(
        nc, in_maps, core_ids=list(range(N_CORES)), trace=trace, **trace_kwargs
    )
    out = np.concatenate([res.results[i]["out"] for i in range(N_CORES)], axis=0)
    return out.astype(np.float32, copy=False), res


def kernel(hidden, word_ids, num_tokens=None, **_unused):
    out, _ = run(hidden, word_ids, trace=False)
    return out



# revision 2
# speedup vs baseline: 1.6556x; 1.6556x over previous
"""Segment mean-pool (BERT lattice embedding) Trainium2 Bass kernel.

Full-input contract: kernel(hidden[64,512,768] f32, word_ids[64,512] i32,
num_tokens=400) -> [64,400,768] f32.

Strategy: data-parallel over batch across 8 NeuronCores (8 samples each).
Per sample b the ragged segment mean  out[t] = mean_{s: wid[s]==t} hidden[s]
is computed as a matmul on the PE array with the reciprocal counts folded
into the one-hot matrix (so the matmul directly produces means):

    A_T[s, t] = (word_ids[b, s] == t) * (1 / count[b, word_ids[b, s]])
    psum[t, :] = sum_j A_T[j-chunk].T @ hidden[b, j-chunk]
    out[t, h] = psum[t, h]            pure PSUM->SBUF cast-copy drain

This kernel is memory-bound (22.4 MB f32 I/O per core at ~358 GB/s HBM), so
everything heavy moves in fp16: hidden is cast host-side to fp16 (halves the
input stream), the one-hot is built in fp16, the matmul runs fp16 (full PE
rate + fast-weight-load, vs fp32r's fp32_mode=HIGH slow path), and the
output is written fp16 and upcast host-side. End-to-end error ~4e-4.

Work pruning: word_ids are sorted per sample, so piece-chunk j (128 pieces)
only overlaps a narrow word range. Each (j-chunk, word-chunk) matmul whose
ranges don't intersect (across ALL samples -- the program is shared SPMD)
contributes zeros and is skipped. The incidence is measured from the actual
word_ids at call time and the program is compiled for it (7 of 16 pairs for
uniform data), cached per incidence pattern.

Engine budget per core: DMA 11.2 MB (the roofline), Tensor ~112 matmuls,
ACT drains psum[:, 0:512], DVE builds one-hots (fp16 2x read rate) and
drains psum[:, 512:768]. All DMAs issue on the sync HWDGE ring: inputs
prefetch first in program order, per-sample output drains queue behind
them, which keeps the single ring line-rate busy with zero bubbles.
"""

import numpy as np

B, S, H, T = 64, 512, 768, 400
N_CORES = 8
B_LOC = B // N_CORES  # samples per core
P = 128
J = S // P  # contraction chunks per sample
N0 = 512  # psum bank0 cols (ACT drains); bank1 = H - N0 (DVE drains)
M_CHUNKS = [(0, 128), (128, 128), (256, 128), (384, T - 384)]  # (t0, mw)
NM = len(M_CHUNKS)

# (word-chunk -> piece-chunks that can touch it) for sorted uniform word_ids;
# recomputed from the actual inputs at call time.
DEFAULT_M_JS = ((0, 1), (1, 2), (2, 3), (3,))

_CACHED = {}


def _measure_m_js(wid):
    """Which piece-chunks j intersect word-chunk m, across all samples."""
    m_js = []
    for t0, mw in M_CHUNKS:
        js = []
        for j in range(J):
            w = wid[:, j * P : (j + 1) * P]
            if ((w >= t0) & (w < t0 + mw)).any():
                js.append(j)
        m_js.append(tuple(js))
    return tuple(m_js)


def build_program(m_js=DEFAULT_M_JS):
    """Build + compile the single-core Bass program (same NEFF on all cores)."""
    import concourse.bass as bass  # noqa: F401
    import concourse.mybir as mybir
    import concourse.tile as tile
    from concourse import bacc

    nc = bacc.Bacc(
        "TRN2",
        target_bir_lowering=False,
        debug=False,
        enable_asserts=False,
        num_devices=N_CORES,
    )
    f32 = mybir.dt.float32
    f16 = mybir.dt.float16
    Alu = mybir.AluOpType

    # hidden host-prearranged as [P, B_LOC, J, H] fp16: partition p holds
    # piece s = 128j + p -> 6 KB contiguous per partition per sample.
    hidden_t = nc.dram_tensor(
        "hidden_pbjh", [P, B_LOC, J, H], f16, kind="ExternalInput"
    ).ap()
    # word_ids as fp32 (values < 400 exact): wid_pbj[p, b, j] = wid[b, 128j+p]
    wid_t = nc.dram_tensor("wid_pbj", [P, B_LOC, J], f32, kind="ExternalInput").ap()
    # per-PIECE reciprocal counts: rp_pbj[p, b, j] = 1/count[b, wid[b, 128j+p]]
    rp_t = nc.dram_tensor("rp_pbj", [P, B_LOC, J], f32, kind="ExternalInput").ap()
    out_t = nc.dram_tensor("out", [B_LOC, T, H], f16, kind="ExternalOutput").ap()

    with tile.TileContext(nc) as tc:
        with tc.tile_pool(name="const", bufs=1) as const_pool, \
             tc.tile_pool(name="hidp", bufs=B_LOC) as hid_pool, \
             tc.tile_pool(name="aTp", bufs=3) as aT_pool, \
             tc.tile_pool(name="outp", bufs=3) as out_pool, \
             tc.tile_pool(name="ps0p", bufs=4, space="PSUM") as ps0_pool, \
             tc.tile_pool(name="ps1p", bufs=4, space="PSUM") as ps1_pool:

            # iota over words, fp16 (exact for ints < 2048) -> 2x DVE read rate
            iota_t = const_pool.tile([P, T], f16, name="iota_t")
            nc.gpsimd.iota(
                iota_t,
                pattern=[[1, T]],
                base=0,
                channel_multiplier=0,
                allow_small_or_imprecise_dtypes=True,
            )

            wid_sb = const_pool.tile([P, B_LOC, J], f32, name="wid_sb")
            nc.sync.dma_start(out=wid_sb, in_=wid_t)
            rp_sb = const_pool.tile([P, B_LOC, J], f32, name="rp_sb")
            nc.sync.dma_start(out=rp_sb, in_=rp_t)

            # Prefetch the whole input shard up front (48 KB/partition, fits
            # SBUF). Sample 0 split per j-chunk so compute starts as soon as
            # chunk 0 lands.
            hids = []
            for b in range(B_LOC):
                hid = hid_pool.tile([P, J, H], f16, name=f"hid{b}", tag="hid")
                if b == 0:
                    for j in range(J):
                        nc.sync.dma_start(out=hid[:, j, :], in_=hidden_t[:, b, j, :])
                else:
                    nc.sync.dma_start(out=hid, in_=hidden_t[:, b])
                hids.append(hid)

            for b in range(B_LOC):
                hid = hids[b]
                # one-hot * recip, fused in one DVE pass per chunk
                aT = aT_pool.tile([P, J, T], f16, name="aT", tag="aT")
                for j in range(J):
                    nc.vector.tensor_scalar(
                        aT[:, j, :],
                        iota_t,
                        wid_sb[:, b, j : j + 1],
                        rp_sb[:, b, j : j + 1],
                        op0=Alu.is_equal,
                        op1=Alu.mult,
                    )

                om = out_pool.tile([P, NM, H], f16, name="om", tag="om")
                for mi, (t0, mw) in enumerate(M_CHUNKS):
                    js = m_js[mi]
                    if not js:  # no pieces can hit this word range: zeros
                        nc.vector.memset(om[:mw, mi, :], 0.0)
                        continue
                    ps0 = ps0_pool.tile([P, N0], f32, name="ps0", tag="ps0")
                    ps1 = ps1_pool.tile([P, H - N0], f32, name="ps1", tag="ps1")
                    for k, j in enumerate(js):
                        st, sp = (k == 0), (k == len(js) - 1)
                        wts = aT[:, j, t0 : t0 + mw]
                        # back-to-back matmuls share the stationary operand
                        nc.tensor.matmul(
                            ps0[:mw], wts, hid[:, j, 0:N0], start=st, stop=sp
                        )
                        nc.tensor.matmul(
                            ps1[:mw], wts, hid[:, j, N0:H], start=st, stop=sp
                        )
                    # drain: ACT takes bank0, DVE takes bank1, both cast to fp16
                    nc.scalar.copy(om[:mw, mi, 0:N0], ps0[:mw])
                    nc.vector.tensor_scalar(
                        om[:mw, mi, N0:H], ps1[:mw], 0.0, None, op0=Alu.add
                    )

                # two output DMAs per sample: [128,3,H] covers words 0..383,
                # [16,H] covers 384..399 (no ragged-tail waste)
                nc.sync.dma_start(
                    out=out_t[b, 0:384].rearrange("(m p) h -> p m h", p=P),
                    in_=om[:, 0:3, :],
                )
                nc.sync.dma_start(out=out_t[b, 384:T], in_=om[: T - 384, 3, :])

    nc.compile()
    return nc


def _prep_in_maps(hidden, word_ids):
    hidden = np.ascontiguousarray(np.asarray(hidden), dtype=np.float32).reshape(B, S, H)
    wid = np.ascontiguousarray(np.asarray(word_ids), dtype=np.int32).reshape(B, S)

    # per-piece reciprocal counts rp[b, s] = 1/count[b, wid[b, s]]
    counts = np.zeros((B, T), np.int64)
    np.add.at(counts, (np.repeat(np.arange(B), S), wid.reshape(-1)), 1)
    rp = (1.0 / np.maximum(counts, 1))[np.arange(B)[:, None], wid].astype(np.float32)

    h4 = hidden.reshape(B, J, P, H)  # piece s = 128j + p
    w4 = wid.reshape(B, J, P)
    r4 = rp.reshape(B, J, P)

    in_maps = []
    for i in range(N_CORES):
        sl = slice(i * B_LOC, (i + 1) * B_LOC)
        # [B_LOC, J, P, *] -> [P, B_LOC, J, *]
        hs = np.ascontiguousarray(h4[sl].transpose(2, 0, 1, 3).astype(np.float16))
        ws = np.ascontiguousarray(w4[sl].transpose(2, 0, 1).astype(np.float32))
        rs = np.ascontiguousarray(r4[sl].transpose(2, 0, 1))
        in_maps.append({"hidden_pbjh": hs, "wid_pbj": ws, "rp_pbj": rs})
    return in_maps


def run(hidden, word_ids, trace=False, **trace_kwargs):
    from concourse import bass_utils

    m_js = _measure_m_js(
        np.ascontiguousarray(np.asarray(word_ids), dtype=np.int32).reshape(B, S)
    )
    if m_js not in _CACHED:
        _CACHED[m_js] = build_program(m_js)
    nc = _CACHED[m_js]
    in_maps = _prep_in_maps(hidden, word_ids)
    res = bass_utils.run_bass_kernel_spmd(
        nc, in_maps, core_ids=list(range(N_CORES)), trace=trace, **trace_kwargs
    )
    out = np.concatenate(
        [np.asarray(res.results[i]["out"]) for i in range(N_CORES)], axis=0
    )
    return out.astype(np.float32), res


def kernel(hidden, word_ids, num_tokens=None, **_unused):
    out, _ = run(hidden, word_ids, trace=False)
    return out


# revision 3
# speedup vs baseline: 1.8931x; 1.1435x over previous
"""Segment mean-pool (BERT lattice embedding) Trainium2 Bass kernel.

Full-input contract: kernel(hidden[64,512,768] f32, word_ids[64,512] i32,
num_tokens=400) -> [64,400,768] f32.

Strategy: data-parallel over batch across 8 NeuronCores (8 samples each).
Per sample b the ragged segment mean  out[t] = mean_{s: wid[s]==t} hidden[s]
is computed as a matmul on the PE array with the reciprocal counts folded
into the one-hot matrix (so the matmul directly produces means):

    A_T[s, t] = (word_ids[b, s] == t) * (1 / count[b, word_ids[b, s]])
    psum[t, :] = sum_j A_T[j-chunk].T @ hidden[b, j-chunk]
    out[t, h] = psum[t, h]            pure PSUM->SBUF cast-copy drain

This kernel is memory-bound (22.4 MB f32 I/O per core at ~358 GB/s HBM), so
everything heavy moves in fp16: hidden is cast host-side to fp16 (halves the
input stream), the one-hot is built in fp16, the matmul runs fp16 (full PE
rate + fast-weight-load, vs fp32r's fp32_mode=HIGH slow path), and the
output is written fp16 and upcast host-side. End-to-end error ~4e-4.

Work pruning: word_ids are sorted per sample, so piece-chunk j (128 pieces)
only overlaps a narrow word range. Each (j-chunk, word-chunk) matmul whose
ranges don't intersect (across ALL samples -- the program is shared SPMD)
contributes zeros and is skipped; one-hots are only built over each j's
256-word window. The incidence is measured from the actual word_ids at call
time and the program is compiled for it (7 of 16 pairs for uniform data),
cached per incidence pattern. Word chunks are padded to 4x128 (words
400..511 compare as never-equal -> zeros) so every matmul has full 128-wide
stationary weights.

Engine budget per core: DMA 11.2 MB on one sync-HWDGE ring (the roofline,
~31 us of line time), Tensor ~112 matmuls, ACT drains psum[:, 0:512] in
two-chunk paired instructions, DVE builds windowed one-hots and drains
psum[:, 512:768]. All tiles are 8-deep so nothing recycles mid-stream: the
input prefetch queues first, per-sample output drains queue behind it, and
the ring never goes idle.
"""

import numpy as np

B, S, H, T = 64, 512, 768, 400
N_CORES = 8
B_LOC = B // N_CORES  # samples per core
P = 128
J = S // P  # contraction chunks per sample
N0 = 512  # psum bank0 cols (ACT drains); bank1 = H - N0 (DVE drains)
NM = 4  # word chunks of 128 (words 400..511 are compare-never-equal padding)
WIN = 256  # one-hot window per piece-chunk (covers <= 2 adjacent word chunks)

# (word-chunk -> piece-chunks that can touch it) for sorted uniform word_ids;
# recomputed from the actual inputs at call time.
DEFAULT_M_JS = ((0, 1), (1, 2), (2, 3), (3,))

_CACHED = {}


def _measure_m_js(wid):
    """Which piece-chunks j intersect word-chunk m, across all samples."""
    m_js = []
    for mi in range(NM):
        t0 = mi * P
        js = []
        for j in range(J):
            w = wid[:, j * P : (j + 1) * P]
            if ((w >= t0) & (w < t0 + P)).any():
                js.append(j)
        m_js.append(tuple(js))
    return tuple(m_js)


def _j_windows(m_js):
    """Per piece-chunk one-hot word-window bases (width WIN, 128-aligned)."""
    j_ms = [[mi for mi in range(NM) if j in m_js[mi]] for j in range(J)]
    bases = []
    for j, ms in enumerate(j_ms):
        if not ms:
            bases.append(0)
            continue
        lo, hi = min(ms), max(ms)
        assert (hi - lo + 1) * P <= WIN, f"chunk {j} spans too many word chunks"
        bases.append(lo * P)
    return bases


def build_program(m_js=DEFAULT_M_JS):
    """Build + compile the single-core Bass program (same NEFF on all cores)."""
    import concourse.bass as bass  # noqa: F401
    import concourse.mybir as mybir
    import concourse.tile as tile
    from concourse import bacc

    nc = bacc.Bacc(
        "TRN2",
        target_bir_lowering=False,
        debug=False,
        enable_asserts=False,
        num_devices=N_CORES,
    )
    f32 = mybir.dt.float32
    f16 = mybir.dt.float16
    Alu = mybir.AluOpType
    jbase = _j_windows(m_js)

    # hidden host-prearranged as [P, B_LOC, J, H] fp16: partition p holds
    # piece s = 128j + p -> 6 KB contiguous per partition per sample.
    hidden_t = nc.dram_tensor(
        "hidden_pbjh", [P, B_LOC, J, H], f16, kind="ExternalInput"
    ).ap()
    # word_ids as fp32 (values < 400 exact): wid_pbj[p, b, j] = wid[b, 128j+p]
    wid_t = nc.dram_tensor("wid_pbj", [P, B_LOC, J], f32, kind="ExternalInput").ap()
    # per-PIECE reciprocal counts: rp_pbj[p, b, j] = 1/count[b, wid[b, 128j+p]]
    rp_t = nc.dram_tensor("rp_pbj", [P, B_LOC, J], f32, kind="ExternalInput").ap()
    out_t = nc.dram_tensor("out", [B_LOC, T, H], f16, kind="ExternalOutput").ap()

    with tile.TileContext(nc) as tc:
        with tc.tile_pool(name="const", bufs=1) as const_pool, \
             tc.tile_pool(name="hidp", bufs=B_LOC) as hid_pool, \
             tc.tile_pool(name="aTp", bufs=B_LOC) as aT_pool, \
             tc.tile_pool(name="outp", bufs=B_LOC) as out_pool, \
             tc.tile_pool(name="ps0p", bufs=2, space="PSUM") as ps0_pool, \
             tc.tile_pool(name="ps1p", bufs=2, space="PSUM") as ps1_pool:

            # iota over padded words, fp16 (exact for ints < 2048)
            iota_t = const_pool.tile([P, NM * P], f16, name="iota_t")
            nc.gpsimd.iota(
                iota_t,
                pattern=[[1, NM * P]],
                base=0,
                channel_multiplier=0,
                allow_small_or_imprecise_dtypes=True,
            )

            wid_sb = const_pool.tile([P, B_LOC, J], f32, name="wid_sb")
            nc.sync.dma_start(out=wid_sb, in_=wid_t)
            rp_sb = const_pool.tile([P, B_LOC, J], f32, name="rp_sb")
            nc.sync.dma_start(out=rp_sb, in_=rp_t)

            # Prefetch the whole input shard up front (48 KB/partition, fits
            # SBUF). Sample 0 split per j-chunk so compute starts as soon as
            # chunk 0 lands.
            hids = []
            for b in range(B_LOC):
                hid = hid_pool.tile([P, J, H], f16, name=f"hid{b}", tag="hid")
                if b == 0:
                    for j in range(J):
                        nc.sync.dma_start(out=hid[:, j, :], in_=hidden_t[:, b, j, :])
                else:
                    nc.sync.dma_start(out=hid, in_=hidden_t[:, b])
                hids.append(hid)

            for b in range(B_LOC):
                hid = hids[b]
                # windowed one-hot * recip, fused in one DVE pass per chunk
                aT = aT_pool.tile([P, J, WIN], f16, name="aT", tag="aT")
                for j in range(J):
                    nc.vector.tensor_scalar(
                        aT[:, j, :],
                        iota_t[:, jbase[j] : jbase[j] + WIN],
                        wid_sb[:, b, j : j + 1],
                        rp_sb[:, b, j : j + 1],
                        op0=Alu.is_equal,
                        op1=Alu.mult,
                    )

                om = out_pool.tile([P, NM, H], f16, name="om", tag="om")
                for pair in ((0, 1), (2, 3)):
                    ps0 = ps0_pool.tile([P, 2, N0], f32, name="ps0", tag="ps0")
                    ps1 = ps1_pool.tile([P, 2, H - N0], f32, name="ps1", tag="ps1")
                    for q, mi in enumerate(pair):
                        t0 = mi * P
                        js = m_js[mi]
                        if not js:  # no pieces can hit this word range: zeros
                            nc.vector.memset(om[:, mi, :], 0.0)
                            continue
                        for k, j in enumerate(js):
                            st, sp = (k == 0), (k == len(js) - 1)
                            w0 = t0 - jbase[j]
                            wts = aT[:, j, w0 : w0 + P]
                            # back-to-back matmuls share the stationary operand
                            nc.tensor.matmul(
                                ps0[:, q, :], wts, hid[:, j, 0:N0], start=st, stop=sp
                            )
                            nc.tensor.matmul(
                                ps1[:, q, :], wts, hid[:, j, N0:H], start=st, stop=sp
                            )
                    # paired drain: ACT takes bank0 cols, DVE takes bank1 cols
                    m0 = pair[0]
                    nc.scalar.copy(om[:, m0 : m0 + 2, 0:N0], ps0)
                    nc.vector.tensor_scalar(
                        om[:, m0 : m0 + 2, N0:H], ps1, 0.0, None, op0=Alu.add
                    )

                # two output DMAs per sample: [128,3,H] covers words 0..383,
                # [16,H] covers 384..399 (no ragged-tail waste)
                nc.sync.dma_start(
                    out=out_t[b, 0:384].rearrange("(m p) h -> p m h", p=P),
                    in_=om[:, 0:3, :],
                )
                nc.sync.dma_start(out=out_t[b, 384:T], in_=om[: T - 384, 3, :])

    nc.compile()
    return nc


def _prep_in_maps(hidden, word_ids):
    hidden = np.ascontiguousarray(np.asarray(hidden), dtype=np.float32).reshape(B, S, H)
    wid = np.ascontiguousarray(np.asarray(word_ids), dtype=np.int32).reshape(B, S)

    # per-piece reciprocal counts rp[b, s] = 1/count[b, wid[b, s]]
    counts = np.zeros((B, T), np.int64)
    np.add.at(counts, (np.repeat(np.arange(B), S), wid.reshape(-1)), 1)
    rp = (1.0 / np.maximum(counts, 1))[np.arange(B)[:, None], wid].astype(np.float32)

    h4 = hidden.reshape(B, J, P, H)  # piece s = 128j + p
    w4 = wid.reshape(B, J, P)
    r4 = rp.reshape(B, J, P)

    in_maps = []
    for i in range(N_CORES):
        sl = slice(i * B_LOC, (i + 1) * B_LOC)
        # [B_LOC, J, P, *] -> [P, B_LOC, J, *]
        hs = np.ascontiguousarray(h4[sl].transpose(2, 0, 1, 3).astype(np.float16))
        ws = np.ascontiguousarray(w4[sl].transpose(2, 0, 1).astype(np.float32))
        rs = np.ascontiguousarray(r4[sl].transpose(2, 0, 1))
        in_maps.append({"hidden_pbjh": hs, "wid_pbj": ws, "rp_pbj": rs})
    return in_maps


def run(hidden, word_ids, trace=False, **trace_kwargs):
    from concourse import bass_utils

    m_js = _measure_m_js(
        np.ascontiguousarray(np.asarray(word_ids), dtype=np.int32).reshape(B, S)
    )
    if m_js not in _CACHED:
        _CACHED[m_js] = build_program(m_js)
    nc = _CACHED[m_js]
    in_maps = _prep_in_maps(hidden, word_ids)
    res = bass_utils.run_bass_kernel_spmd(
        nc, in_maps, core_ids=list(range(N_CORES)), trace=trace, **trace_kwargs
    )
    out = np.concatenate(
        [np.asarray(res.results[i]["out"]) for i in range(N_CORES)], axis=0
    )
    return out.astype(np.float32), res


def kernel(hidden, word_ids, num_tokens=None, **_unused):
    out, _ = run(hidden, word_ids, trace=False)
    return out
